# revision 1
# baseline (speedup 1.0000x reference)
"""Fused transformer block (nn_Block_2388001816768) on 8 Trainium2 NeuronCores.

Sharding: (batch, sequence-half) -> one core. Core c handles batch c//2,
query rows [o*1024:(o+1)*1024] where o = c%2. No collectives: each core
recomputes LN1 + K/V projections for the full sequence of its batch.

Per-core local sequence order is [own half | other half]; causality then
becomes: local triangle over half-1 (identical structure on every core,
handled at tile granularity + a tril constant on diagonal tiles), and
half-2 all-or-nothing (handled by a per-core additive bias in the exp).

All large matmuls run in float32r (TF32-like, full PE rate), fp32 accum.
LN scale/shift (g, b) are folded into the projection weights host-side;
the 1/sqrt(HD) score scale is folded into Wq/bq.
"""

import os

import numpy as np

import concourse.bacc as bacc
import concourse.bass as bass  # noqa: F401
import concourse.mybir as mybir
import concourse.tile as tile
from concourse.bass_utils import run_bass_kernel_spmd
from concourse.masks import make_identity

B, T, D, H = 4, 2048, 1024, 16
HD = D // H  # 64
FF = 4 * D  # 4096
TQ = T // 2  # rows per core = 1024
P = 128
NEG = -60000.0  # additive mask: exp(x + NEG) == 0 in fp32

f32 = mybir.dt.float32
f32r = mybir.dt.float32r
AF = mybir.ActivationFunctionType
ALU = mybir.AluOpType

_CACHE = {}


def _build_program():
    nc = bacc.Bacc(None, target_bir_lowering=False)

    xl_d = nc.dram_tensor("xl", (T, D), f32, kind="ExternalInput")
    wq_d = nc.dram_tensor("wq", (D, D), f32r, kind="ExternalInput")
    wk_d = nc.dram_tensor("wk", (D, D), f32r, kind="ExternalInput")
    wv_d = nc.dram_tensor("wv", (D, D), f32r, kind="ExternalInput")
    wo_d = nc.dram_tensor("wo", (D, D), f32r, kind="ExternalInput")
    w1_d = nc.dram_tensor("w1", (D, FF), f32r, kind="ExternalInput")
    w2_d = nc.dram_tensor("w2", (FF, D), f32r, kind="ExternalInput")
    qkvb_d = nc.dram_tensor("qkvb", (P, 3 * H // 2), f32, kind="ExternalInput")
    bo_d = nc.dram_tensor("bo_", (1, D), f32, kind="ExternalInput")
    b1f_d = nc.dram_tensor("b1f", (P, FF // P), f32, kind="ExternalInput")
    b2_d = nc.dram_tensor("b2_", (1, D), f32, kind="ExternalInput")
    h2b_d = nc.dram_tensor("h2b", (1, 1), f32, kind="ExternalInput")
    out_d = nc.dram_tensor("out", (TQ, D), f32, kind="ExternalOutput")

    DT = D // P  # 8 d-tiles
    NT = T // P  # 16 t-tiles
    NQ = TQ // P  # 8 q-tiles
    FT = FF // P  # 32 ff-tiles

    KDBG = os.environ.get("KDBG", "0") == "1"
    if KDBG:
        dbg_hT_d = nc.dram_tensor("dbg_hT", (P, DT, T), f32, kind="ExternalOutput")
        dbg_qT_d = nc.dram_tensor("dbg_qT", (HD, TQ), f32, kind="ExternalOutput")
        dbg_kT_d = nc.dram_tensor("dbg_kT", (HD, T), f32, kind="ExternalOutput")
        dbg_va_d = nc.dram_tensor("dbg_va", (P, NT, HD + 1), f32, kind="ExternalOutput")
        dbg_pt_d = nc.dram_tensor("dbg_pt", (2, P, 512), f32, kind="ExternalOutput")
        dbg_cx_d = nc.dram_tensor("dbg_cx", (HD + 1, 512), f32, kind="ExternalOutput")
        dbg_bc_d = nc.dram_tensor("dbg_bc", (HD, TQ), f32, kind="ExternalOutput")
        dbg_ct_d = nc.dram_tensor("dbg_ct", (P, DT, TQ), f32, kind="ExternalOutput")
        dbg_x2_d = nc.dram_tensor("dbg_x2", (NQ, P, D), f32, kind="ExternalOutput")

    with tile.TileContext(nc) as tc:
        with (
            tc.tile_pool(name="const", bufs=1) as const,
            tc.tile_pool(name="dramp", bufs=1, space="DRAM") as dramp,
        ):
            ident_f = const.tile([P, P], f32)
            make_identity(nc, ident_f)
            ident = const.tile([P, P], f32r)
            nc.vector.tensor_copy(ident, ident_f)
            # S^T-space causal keep mask: keep where kv(part) <= q(free)
            tril_f = const.tile([P, P], f32)
            nc.gpsimd.memset(tril_f, 1.0)
            nc.gpsimd.affine_select(
                out=tril_f, in_=tril_f, compare_op=ALU.is_ge, fill=0.0,
                base=0, pattern=[[1, P]], channel_multiplier=-1,
            )
            tril = const.tile([P, P], f32r)
            nc.vector.tensor_copy(tril, tril_f)
            ones16 = const.tile([P, NT], f32)
            nc.vector.memset(ones16, 1.0)
            qkvb = const.tile([P, 3 * H // 2], f32)
            nc.sync.dma_start(qkvb, qkvb_d[:, :])
            eps = const.tile([P, 1], f32)
            nc.vector.memset(eps, 1e-5)
            h2b = const.tile([P, 1], f32)
            nc.sync.dma_start(h2b, h2b_d.ap().to_broadcast([P, 1]))

            x2d = dramp.tile([NQ, P, D], f32)  # x2 spill (post-attn residual)

            with tc.tile_pool(name="ctxp", bufs=1) as ctxp:
                ctxT = ctxp.tile([P, DT, TQ], f32r)  # ctx^T head-pair-stacked

                with tc.tile_pool(name="hTp", bufs=1) as hTp:
                    # h^T in 4 t-chunks of 512 so phase 2 can overlap phase 1
                    hTc = []
                    for i in range(4):
                        hT_i = hTp.tile([P, DT, 512], f32r, tag=f"hT{i}")
                        hTc.append(hT_i)

                    # ---------- Phase 1: LN1 + transpose ----------
                    with (
                        tc.tile_pool(name="ln1", bufs=5) as ln1,
                        tc.tile_pool(name="ps1", bufs=4, space="PSUM") as ps1,
                    ):
                        for tt in range(NT):
                            x_t = ln1.tile([P, D], f32, tag="x_t")
                            nc.sync.dma_start(x_t, xl_d[tt * P:(tt + 1) * P, :])
                            st = ln1.tile([P, 2, 6], f32, tag="st")
                            nc.vector.bn_stats(st[:, 0, :], x_t[:, 0:512])
                            nc.vector.bn_stats(st[:, 1, :], x_t[:, 512:1024])
                            mv = ln1.tile([P, 2], f32, tag="mv")
                            nc.vector.bn_aggr(mv, st)
                            rstd = ln1.tile([P, 1], f32, tag="rstd")
                            nc.scalar.activation(rstd, mv[:, 1:2], AF.Sqrt, bias=eps)
                            nc.vector.reciprocal(rstd, rstd)
                            nb = ln1.tile([P, 2], f32, tag="nb")
                            nc.vector.tensor_scalar_mul(nb[:, 0:1], rstd, -1.0)
                            nc.vector.tensor_mul(
                                nb[:, 1:2], mv[:, 0:1], nb[:, 0:1])
                            h_t = ln1.tile([P, D], f32r, tag="h_t")
                            tpos = tt % 4
                            for dh in range(2):
                                hsl = slice(dh * 512, (dh + 1) * 512)
                                nc.scalar.activation(
                                    h_t[:, hsl], x_t[:, hsl], AF.Identity,
                                    bias=nb[:, 1:2], scale=rstd)
                                tp = ps1.tile([P, 4, P], f32r, tag="tp")
                                for k in range(4):
                                    dt = dh * 4 + k
                                    nc.tensor.transpose(
                                        tp[:, k, :],
                                        h_t[:, dt * P:(dt + 1) * P], ident)
                                dst = hTc[tt // 4][:, dh * 4:dh * 4 + 4,
                                                   tpos * P:(tpos + 1) * P]
                                if dh == 0:
                                    nc.scalar.copy(dst, tp)
                                else:
                                    nc.vector.tensor_copy(dst, tp)

                    if KDBG:
                        pass

                    # ---------- Phase 2: head-pair QKV + attention ----------
                    # Heads processed in pairs: even head in partitions 0:64,
                    # odd head in 64:128 of shared pair tiles; projections use
                    # tile_position col-split, scores use row-split so the two
                    # K=64 matmuls run concurrently in the PE array.
                    with (
                        tc.tile_pool(name="whead", bufs=1) as whead,
                        tc.tile_pool(name="head", bufs=2) as head,
                        tc.tile_pool(name="pt", bufs=3) as ptp,
                        tc.tile_pool(name="psqkv", bufs=2, space="PSUM") as psqkv,
                        tc.tile_pool(name="pssc", bufs=4, space="PSUM") as pssc,
                        tc.tile_pool(name="psctx", bufs=1, space="PSUM") as psctx,
                    ):
                        for hp in range(H // 2):
                            wp = whead.tile([P, 3, DT, 2 * HD], f32r, tag="wp")
                            for wi, w_dram in enumerate((wq_d, wk_d, wv_d)):
                                nc.sync.dma_start(
                                    wp[:, wi],
                                    w_dram[:, hp * 2 * HD:(hp + 1) * 2 * HD]
                                    .rearrange("(dt q) m -> q dt m", q=P))

                            qT = head.tile([P, TQ], f32r, tag="qT")
                            kT = head.tile([P, T], f32r, tag="kT")
                            vT = head.tile([P, T], f32r, tag="vT")
                            vaug_e = head.tile([P, NT, HD + 1], f32r, tag="vaug_e")
                            vaug_o = head.tile([P, NT, HD + 1], f32r, tag="vaug_o")
                            nc.vector.tensor_copy(
                                vaug_e[:, :, HD:HD + 1], ones16.unsqueeze(2))
                            nc.vector.tensor_copy(
                                vaug_o[:, :, HD:HD + 1], ones16.unsqueeze(2))
                            for (wi, dst, nchunk, bcol) in (
                                (0, qT, TQ // 512, 3 * hp + 0),
                                (1, kT, T // 512, 3 * hp + 1),
                                (2, vT, T // 512, 3 * hp + 2),
                            ):
                                for c in range(nchunk):
                                    pp = psqkv.tile([P, 512], f32, tag="pp")
                                    for dt in range(DT):
                                        nc.tensor.matmul(
                                            pp, wp[:, wi, dt, :],
                                            hTc[c][:, dt, :],
                                            start=(dt == 0), stop=(dt == DT - 1))
                                    nc.vector.tensor_scalar_add(
                                        out=dst[:, c * 512:(c + 1) * 512],
                                        in0=pp, scalar1=qkvb[:, bcol:bcol + 1])
                                    if wi == 2:  # transpose V per chunk, inline
                                        for kt in range(4 * c, 4 * c + 4):
                                            ksl = slice(kt * P, (kt + 1) * P)
                                            vp = pssc.tile([P, 512], f32r,
                                                           tag="sps")
                                            nc.tensor.transpose(
                                                vp[:, 0:P], vT[:, ksl], ident)
                                            nc.vector.tensor_copy(
                                                vaug_e[:, kt, 0:HD], vp[:, 0:HD])
                                            nc.vector.tensor_copy(
                                                vaug_o[:, kt, 0:HD], vp[:, HD:P])


                            for qc in range(2):
                                qsl = slice(qc * 512, (qc + 1) * 512)
                                kv_tiles = (list(range(0, (qc + 1) * 4))
                                            + list(range(8, 16)))
                                ctx_e = psctx.tile([HD + 1, 512], f32, tag="ctx_e")
                                ctx_o = psctx.tile([HD + 1, 512], f32, tag="ctx_o")
                                for n, i in enumerate(kv_tiles):
                                    isl = slice(i * P, (i + 1) * P)
                                    sps_e = pssc.tile([P, 512], f32, tag="sps")
                                    nc.tensor.matmul(
                                        sps_e, kT[0:HD, isl], qT[0:HD, qsl],
                                        start=True, stop=True,
                                        tile_position=(0, 0))
                                    sps_o = pssc.tile([P, 512], f32, tag="sps")
                                    nc.tensor.matmul(
                                        sps_o, kT[HD:P, isl], qT[HD:P, qsl],
                                        start=True, stop=True,
                                        tile_position=(HD, 0))
                                    for sps, vaug, ctx in (
                                        (sps_e, vaug_e, ctx_e),
                                        (sps_o, vaug_o, ctx_o),
                                    ):
                                        pt = ptp.tile([P, 512], f32r, tag="pt")
                                        if i >= 8:  # other half: all-or-nothing
                                            nc.scalar.activation(
                                                pt, sps, AF.Exp, bias=h2b)
                                        else:
                                            jd = i - qc * 4  # diag col subtile
                                            if jd < 0:  # fully visible
                                                nc.scalar.activation(pt, sps, AF.Exp)
                                            else:
                                                if jd > 0:
                                                    nc.vector.memset(
                                                        pt[:, 0:jd * P].bitcast(f32),
                                                        0.0)
                                                dsl = slice(jd * P, (jd + 1) * P)
                                                nc.scalar.activation(
                                                    pt[:, dsl], sps[:, dsl], AF.Exp)
                                                nc.vector.tensor_mul(
                                                    pt[:, dsl], pt[:, dsl], tril)
                                                if jd < 3:
                                                    rsl = slice((jd + 1) * P, 512)
                                                    nc.scalar.activation(
                                                        pt[:, rsl], sps[:, rsl],
                                                        AF.Exp)
                                        nc.tensor.matmul(
                                            ctx, vaug[:, i, :], pt,
                                            start=(n == 0),
                                            stop=(n == len(kv_tiles) - 1))
                                # softmax normalization per head
                                for hh, ctx in ((0, ctx_e), (1, ctx_o)):
                                    rr = ptp.tile([HD + 1, 512], f32, tag="rr")
                                    rr0 = ptp.tile([1, 512], f32, tag="rr0")
                                    bc = ptp.tile([HD, 512], f32, tag="bc")
                                    nc.vector.reciprocal(
                                        rr[HD:HD + 1, :], ctx[HD:HD + 1, :])
                                    nc.sync.dma_start(rr0, rr[HD:HD + 1, :])
                                    nc.gpsimd.partition_broadcast(bc, rr0)
                                    if hh == 0:
                                        nc.vector.tensor_mul(
                                            ctxT[0:HD, hp, qsl], ctx[0:HD, :], bc)
                                    else:
                                        tmp = ptp.tile([HD, 512], f32r, tag="tmp")
                                        nc.vector.tensor_mul(tmp, ctx[0:HD, :], bc)
                                        nc.sync.dma_start(ctxT[HD:P, hp, qsl], tmp)

                if KDBG:
                    nc.sync.dma_start(dbg_ct_d.ap(), ctxT[:, :, :].bitcast(f32))

                # ---------- Phase 3: Wo proj + residual -> x2 (DRAM) ----------
                with (
                    tc.tile_pool(name="wop", bufs=1) as wop,
                    tc.tile_pool(name="ph3", bufs=3) as ph3,
                    tc.tile_pool(name="ps3", bufs=2, space="PSUM") as ps3,
                ):
                    wo_sb = wop.tile([P, DT, D], f32r)
                    nc.sync.dma_start(
                        wo_sb, wo_d.ap().rearrange("(pc p) n -> p pc n", p=P))
                    bo_bc = wop.tile([P, D], f32)
                    nc.sync.dma_start(bo_bc, bo_d.ap().to_broadcast([P, D]))
                    for qt in range(NQ):
                        xo_t = ph3.tile([P, D], f32, tag="xo_t")
                        nc.sync.dma_start(xo_t, xl_d[qt * P:(qt + 1) * P, :])
                        x2_t = ph3.tile([P, D], f32, tag="x2_t")
                        for dc in range(2):
                            dsl = slice(dc * 512, (dc + 1) * 512)
                            acc = ps3.tile([P, 512], f32, tag="acc")
                            for pc in range(DT):
                                nc.tensor.matmul(
                                    acc, ctxT[:, pc, qt * P:(qt + 1) * P],
                                    wo_sb[:, pc, dsl],
                                    start=(pc == 0), stop=(pc == DT - 1))
                            nc.vector.tensor_add(x2_t[:, dsl], acc, xo_t[:, dsl])
                            nc.vector.tensor_add(
                                x2_t[:, dsl], x2_t[:, dsl], bo_bc[:, dsl])
                        nc.sync.dma_start(x2d[qt], x2_t)
                        if KDBG:
                            nc.sync.dma_start(dbg_x2_d[qt], x2_t)

            # ---------- Phase 4: LN2 + transpose ----------
            with tc.tile_pool(name="h2Tp", bufs=1) as h2Tp:
                h2Tc = []
                for i in range(2):
                    h2T_i = h2Tp.tile([P, DT, 512], f32r, tag=f"h2T{i}")
                    h2Tc.append(h2T_i)
                with (
                    tc.tile_pool(name="ln2", bufs=5) as ln2,
                    tc.tile_pool(name="ps4", bufs=4, space="PSUM") as ps4,
                ):
                    for qt in range(NQ):
                        x2_t = ln2.tile([P, D], f32, tag="x2_t")
                        nc.sync.dma_start(x2_t, x2d[qt])
                        st = ln2.tile([P, 2, 6], f32, tag="st")
                        nc.vector.bn_stats(st[:, 0, :], x2_t[:, 0:512])
                        nc.vector.bn_stats(st[:, 1, :], x2_t[:, 512:1024])
                        mv = ln2.tile([P, 2], f32, tag="mv")
                        nc.vector.bn_aggr(mv, st)
                        rstd = ln2.tile([P, 1], f32, tag="rstd")
                        nc.scalar.activation(rstd, mv[:, 1:2], AF.Sqrt, bias=eps)
                        nc.vector.reciprocal(rstd, rstd)
                        nb = ln2.tile([P, 2], f32, tag="nb")
                        nc.vector.tensor_scalar_mul(nb[:, 0:1], rstd, -1.0)
                        nc.vector.tensor_mul(nb[:, 1:2], mv[:, 0:1], nb[:, 0:1])
                        h2_t = ln2.tile([P, D], f32r, tag="h2_t")
                        qpos = qt % 4
                        for dh in range(2):
                            hsl = slice(dh * 512, (dh + 1) * 512)
                            nc.scalar.activation(
                                h2_t[:, hsl], x2_t[:, hsl], AF.Identity,
                                bias=nb[:, 1:2], scale=rstd)
                            tp = ps4.tile([P, 4, P], f32r, tag="tp")
                            for k in range(4):
                                dt = dh * 4 + k
                                nc.tensor.transpose(
                                    tp[:, k, :],
                                    h2_t[:, dt * P:(dt + 1) * P], ident)
                            dst = h2Tc[qt // 4][:, dh * 4:dh * 4 + 4,
                                                qpos * P:(qpos + 1) * P]
                            if dh == 0:
                                nc.scalar.copy(dst, tp)
                            else:
                                nc.vector.tensor_copy(dst, tp)

                # ---------- Phase 5: FFN ----------
                with (
                    tc.tile_pool(name="ffcst", bufs=1) as ffcp,
                    tc.tile_pool(name="ffw", bufs=5) as ffw,
                    tc.tile_pool(name="g1p", bufs=1) as g1p,
                    tc.tile_pool(name="ffo", bufs=3) as ffo,
                    tc.tile_pool(name="psa", bufs=3, space="PSUM") as psa,
                    tc.tile_pool(name="psf", bufs=1, space="PSUM") as psf,
                ):
                    b1f_sb = ffcp.tile([P, FT], f32)
                    nc.sync.dma_start(b1f_sb, b1f_d[:, :])
                    b2_bc = ffcp.tile([P, D], f32)
                    nc.sync.dma_start(b2_bc, b2_d.ap().to_broadcast([P, D]))
                    g1 = g1p.tile([P, FT, 512], f32r)
                    for qc in range(2):
                        qsl = slice(qc * 512, (qc + 1) * 512)
                        # W1 + GELU for this q chunk, all ff chunks
                        for fc in range(FT):
                            w1c = ffw.tile([P, DT, P], f32r, tag="w1c")
                            nc.sync.dma_start(
                                w1c, w1_d[:, fc * P:(fc + 1) * P].rearrange(
                                    "(dt p) m -> p dt m", p=P))
                            aps = psa.tile([P, 512], f32, tag="aps")
                            for dt in range(DT):
                                nc.tensor.matmul(
                                    aps, w1c[:, dt, :], h2Tc[qc][:, dt, :],
                                    start=(dt == 0), stop=(dt == DT - 1))
                            nc.scalar.activation(
                                g1[:, fc, :], aps, AF.Gelu,
                                bias=b1f_sb[:, fc:fc + 1])
                        # W2 for this q chunk
                        for dh in range(2):
                            dsl = slice(dh * 512, (dh + 1) * 512)
                            fps = []
                            for j in range(4):
                                fps_j = psf.tile([P, 512], f32, tag=f"fps{j}")
                                fps.append(fps_j)
                            for fc in range(FT):
                                w2c = ffw.tile([P, 512], f32r, tag="w2c")
                                nc.sync.dma_start(
                                    w2c, w2_d[fc * P:(fc + 1) * P, dsl])
                                for j in range(4):
                                    nc.tensor.matmul(
                                        fps[j], g1[:, fc, j * P:(j + 1) * P],
                                        w2c, start=(fc == 0),
                                        stop=(fc == FT - 1))
                            for j in range(4):
                                qt = qc * 4 + j
                                o_t = ffo.tile([P, 512], f32, tag="o_t")
                                x2s = ffo.tile([P, 512], f32, tag="x2s")
                                nc.sync.dma_start(x2s, x2d[qt, :, dsl])
                                nc.vector.tensor_add(o_t, fps[j], x2s)
                                nc.vector.tensor_add(o_t, o_t, b2_bc[:, dsl])
                                nc.sync.dma_start(
                                    out_d[qt * P:(qt + 1) * P, dsl], o_t)

    nc.compile()
    return nc


def _prep_inputs(inputs):
    """Host-side: fold LN affine + score scale into weights; build per-core maps."""
    x = np.asarray(inputs["x"], dtype=np.float32)
    g1, b1_ = np.asarray(inputs["ln1_g"], np.float32), np.asarray(inputs["ln1_b"], np.float32)
    g2, b2_ = np.asarray(inputs["ln2_g"], np.float32), np.asarray(inputs["ln2_b"], np.float32)
    Wq = np.asarray(inputs["Wq"], np.float32)  # [H, D, HD]
    Wk = np.asarray(inputs["Wk"], np.float32)
    Wv = np.asarray(inputs["Wv"], np.float32)
    bq = np.asarray(inputs["bq"], np.float32)  # [H, HD]
    bk = np.asarray(inputs["bk"], np.float32)
    bv = np.asarray(inputs["bv"], np.float32)
    Wo = np.asarray(inputs["Wo"], np.float32)
    bo = np.asarray(inputs["bo"], np.float32)
    W1 = np.asarray(inputs["W1"], np.float32)
    b1 = np.asarray(inputs["b1"], np.float32)
    W2 = np.asarray(inputs["W2"], np.float32)
    b2 = np.asarray(inputs["b2"], np.float32)

    sc = 1.0 / np.sqrt(np.float32(HD))
    # [H, D, HD] -> [D, H*HD]
    wq_flat = np.transpose(Wq, (1, 0, 2)).reshape(D, D)
    wk_flat = np.transpose(Wk, (1, 0, 2)).reshape(D, D)
    wv_flat = np.transpose(Wv, (1, 0, 2)).reshape(D, D)
    wq_f = (g1[:, None] * wq_flat) * sc
    wk_f = g1[:, None] * wk_flat
    wv_f = g1[:, None] * wv_flat
    bq_f = (b1_ @ wq_flat + bq.reshape(D)) * sc
    bk_f = b1_ @ wk_flat + bk.reshape(D)
    bv_f = b1_ @ wv_flat + bv.reshape(D)
    # qkvb[0:64, 3*p+w] = even-head bias, [64:128, 3*p+w] = odd-head bias
    qkvb = np.zeros((P, 3 * H // 2), np.float32)
    for p_ in range(H // 2):
        for wi, bf in enumerate((bq_f, bk_f, bv_f)):
            qkvb[0:HD, 3 * p_ + wi] = bf[(2 * p_) * HD:(2 * p_ + 1) * HD]
            qkvb[HD:P, 3 * p_ + wi] = bf[(2 * p_ + 1) * HD:(2 * p_ + 2) * HD]

    w1_f = g2[:, None] * W1
    b1_f = (b2_ @ W1 + b1).reshape(FF // P, P).T.copy()  # [P, FF//P]

    shared = {
        "wq": np.ascontiguousarray(wq_f), "wk": np.ascontiguousarray(wk_f),
        "wv": np.ascontiguousarray(wv_f), "wo": np.ascontiguousarray(Wo),
        "w1": np.ascontiguousarray(w1_f), "w2": np.ascontiguousarray(W2),
        "qkvb": qkvb, "bo_": bo.reshape(1, D),
        "b1f": np.ascontiguousarray(b1_f), "b2_": b2.reshape(1, D),
    }
    in_maps = []
    for c in range(8):
        b, o = c // 2, c % 2
        own = x[b, o * TQ:(o + 1) * TQ]
        oth = x[b, (1 - o) * TQ:(2 - o) * TQ]
        m = dict(shared)
        m["xl"] = np.ascontiguousarray(np.concatenate([own, oth], axis=0))
        m["h2b"] = np.array([[0.0 if o == 1 else NEG]], np.float32)
        in_maps.append(m)
    return in_maps


def kernel(**inputs):
    if "nc" not in _CACHE:
        _CACHE["nc"] = _build_program()
    nc = _CACHE["nc"]
    in_maps = _prep_inputs(inputs)
    res = run_bass_kernel_spmd(nc, in_maps, core_ids=list(range(8)))
    out = np.empty((B, T, D), np.float32)
    for c in range(8):
        b, o = c // 2, c % 2
        out[b, o * TQ:(o + 1) * TQ] = res.results[c]["out"]
    return out



# revision 8
# speedup vs baseline: 43.3619x; 43.3619x over previous
"""Fused transformer block (nn_Block_2388001816768) on 8 Trainium2 NeuronCores.

Sharding: (batch, sequence-half) -> one core. Core c handles batch c//2,
query rows [o*1024:(o+1)*1024] where o = c%2. No collectives: each core
recomputes LN1 + K/V projections for the full sequence of its batch.

Per-core local sequence order is [own half | other half]; causality then
becomes: local triangle over half-1 (identical structure on every core,
handled at tile granularity + a tril constant on diagonal tiles), and
half-2 all-or-nothing (handled by a per-core additive bias in the exp).

All large matmuls run in float32r (TF32-like, full PE rate), fp32 accum.
LN scale/shift (g, b) are folded into the projection weights host-side;
the 1/sqrt(HD) score scale is folded into Wq/bq.
"""

import os

import numpy as np

import concourse.bacc as bacc
import concourse.bass as bass  # noqa: F401
import concourse.mybir as mybir
import concourse.tile as tile
from concourse.bass_utils import run_bass_kernel_spmd
from concourse.masks import make_identity

B, T, D, H = 4, 2048, 1024, 16
HD = D // H  # 64
FF = 4 * D  # 4096
TQ = T // 2  # rows per core = 1024
P = 128
NEG = -60000.0  # additive mask: exp(x + NEG) == 0 in fp32

f32 = mybir.dt.float32
f32r = mybir.dt.float32r
fp8 = mybir.dt.float8e4
AF = mybir.ActivationFunctionType
ALU = mybir.AluOpType
DR = mybir.MatmulPerfMode.DoubleRow
W1S = 16.0  # host-side weight scale (fp8 range), undone by activation scale
W2S = 64.0

_CACHE = {}


def _emit_body(nc, tc, sfx, cst, x2d, dram):
    (xl_d, wq_d, wk_d, wv_d, wo_d, w1_d, w2_d, bo_d, b1f_d, b2_d, out_d) = dram
    ident, tril, ones16, qkvb, eps, h2b = cst

    DT = D // P  # 8 d-tiles
    NT = T // P  # 16 t-tiles
    NQ = TQ // P  # 8 q-tiles
    FT = FF // P  # 32 ff-tiles

    with tc.tile_pool(name="ctxp" + sfx, bufs=1) as ctxp:
        ctxT = ctxp.tile([P, DT, TQ], f32r)  # ctx^T head-pair-stacked

        with tc.tile_pool(name="hTp" + sfx, bufs=1) as hTp:
            # h^T in 4 t-chunks of 512 so phase 2 can overlap phase 1
            hTc = []
            for i in range(4):
                hT_i = hTp.tile([P, DT, 512], f32r, tag=f"hT{i}")
                hTc.append(hT_i)

            # ---------- Phase 1: LN1 + transpose ----------
            with (
                tc.tile_pool(name="ln1" + sfx, bufs=5) as ln1,
                tc.tile_pool(name="ps1" + sfx, bufs=4, space="PSUM") as ps1,
            ):
                for tt in range(NT):
                    x_t = ln1.tile([P, D], f32, tag="x_t")
                    nc.sync.dma_start(x_t, xl_d[tt * P:(tt + 1) * P, :])
                    st = ln1.tile([P, 2, 6], f32, tag="st")
                    nc.vector.bn_stats(st[:, 0, :], x_t[:, 0:512])
                    nc.vector.bn_stats(st[:, 1, :], x_t[:, 512:1024])
                    mv = ln1.tile([P, 2], f32, tag="mv")
                    nc.vector.bn_aggr(mv, st)
                    rstd = ln1.tile([P, 1], f32, tag="rstd")
                    nc.scalar.activation(rstd, mv[:, 1:2], AF.Sqrt, bias=eps)
                    nc.vector.reciprocal(rstd, rstd)
                    nb = ln1.tile([P, 2], f32, tag="nb")
                    nc.vector.tensor_scalar_mul(nb[:, 0:1], rstd, -1.0)
                    nc.vector.tensor_mul(
                        nb[:, 1:2], mv[:, 0:1], nb[:, 0:1])
                    h_t = ln1.tile([P, D], f32r, tag="h_t")
                    tpos = tt % 4
                    for dh in range(2):
                        hsl = slice(dh * 512, (dh + 1) * 512)
                        nc.scalar.activation(
                            h_t[:, hsl], x_t[:, hsl], AF.Identity,
                            bias=nb[:, 1:2], scale=rstd)
                        tp = ps1.tile([P, 4, P], f32r, tag="tp")
                        for k in range(4):
                            dt = dh * 4 + k
                            nc.tensor.transpose(
                                tp[:, k, :],
                                h_t[:, dt * P:(dt + 1) * P], ident)
                        dst = hTc[tt // 4][:, dh * 4:dh * 4 + 4,
                                           tpos * P:(tpos + 1) * P]
                        if dh == 0:
                            nc.scalar.copy(dst, tp)
                        else:
                            nc.vector.tensor_copy(dst, tp)

            # ---------- Phase 2: head-pair QKV + attention ----------
            # Heads processed in pairs: even head in partitions 0:64,
            # odd head in 64:128 of shared pair tiles; projections use
            # tile_position col-split, scores use row-split so the two
            # K=64 matmuls run concurrently in the PE array.
            with (
                tc.tile_pool(name="whead" + sfx, bufs=1) as whead,
                tc.tile_pool(name="head" + sfx, bufs=2) as head,
                tc.tile_pool(name="pt" + sfx, bufs=3) as ptp,
                tc.tile_pool(name="psqkv" + sfx, bufs=2, space="PSUM") as psqkv,
                tc.tile_pool(name="pssc" + sfx, bufs=4, space="PSUM") as pssc,
                tc.tile_pool(name="psctx" + sfx, bufs=1, space="PSUM") as psctx,
            ):
                for hp in range(H // 2):
                    wp = whead.tile([P, 3, DT, 2 * HD], f32r, tag="wp")
                    for wi, w_dram in enumerate((wq_d, wk_d, wv_d)):
                        nc.sync.dma_start(
                            wp[:, wi],
                            w_dram[:, hp * 2 * HD:(hp + 1) * 2 * HD]
                            .rearrange("(dt q) m -> q dt m", q=P))

                    qT = head.tile([P, TQ], f32r, tag="qT")
                    kT = head.tile([P, T], f32r, tag="kT")
                    vT = head.tile([P, T], f32r, tag="vT")
                    vaug_e = head.tile([P, NT, HD + 1], f32r, tag="vaug_e")
                    vaug_o = head.tile([P, NT, HD + 1], f32r, tag="vaug_o")
                    nc.vector.tensor_copy(
                        vaug_e[:, :, HD:HD + 1], ones16.unsqueeze(2))
                    nc.vector.tensor_copy(
                        vaug_o[:, :, HD:HD + 1], ones16.unsqueeze(2))
                    for (wi, dst, nchunk, bcol) in (
                        (0, qT, TQ // 512, 3 * hp + 0),
                        (1, kT, T // 512, 3 * hp + 1),
                        (2, vT, T // 512, 3 * hp + 2),
                    ):
                        for c in range(nchunk):
                            pp = psqkv.tile([P, 512], f32, tag="pp")
                            for dt in range(DT):
                                nc.tensor.matmul(
                                    pp, wp[:, wi, dt, :],
                                    hTc[c][:, dt, :],
                                    start=(dt == 0), stop=(dt == DT - 1))
                            nc.vector.tensor_scalar_add(
                                out=dst[:, c * 512:(c + 1) * 512],
                                in0=pp, scalar1=qkvb[:, bcol:bcol + 1])
                            if wi == 2:  # transpose V per chunk, inline
                                for kt in range(4 * c, 4 * c + 4):
                                    ksl = slice(kt * P, (kt + 1) * P)
                                    vp = pssc.tile([P, 512], f32r,
                                                   tag="sps")
                                    nc.tensor.transpose(
                                        vp[:, 0:P], vT[:, ksl], ident)
                                    nc.vector.tensor_copy(
                                        vaug_e[:, kt, 0:HD], vp[:, 0:HD])
                                    nc.vector.tensor_copy(
                                        vaug_o[:, kt, 0:HD], vp[:, HD:P])

                    for qc in range(2):
                        qsl = slice(qc * 512, (qc + 1) * 512)
                        kv_tiles = (list(range(0, (qc + 1) * 4))
                                    + list(range(8, 16)))
                        ctx_e = psctx.tile([HD + 1, 512], f32, tag="ctx_e")
                        ctx_o = psctx.tile([HD + 1, 512], f32, tag="ctx_o")
                        for n, i in enumerate(kv_tiles):
                            isl = slice(i * P, (i + 1) * P)
                            sps_e = pssc.tile([P, 512], f32, tag="sps")
                            nc.tensor.matmul(
                                sps_e, kT[0:HD, isl], qT[0:HD, qsl],
                                start=True, stop=True,
                                tile_position=(0, 0))
                            sps_o = pssc.tile([P, 512], f32, tag="sps")
                            nc.tensor.matmul(
                                sps_o, kT[HD:P, isl], qT[HD:P, qsl],
                                start=True, stop=True,
                                tile_position=(HD, 0))
                            for sps, vaug, ctx in (
                                (sps_e, vaug_e, ctx_e),
                                (sps_o, vaug_o, ctx_o),
                            ):
                                pt = ptp.tile([P, 512], f32r, tag="pt")
                                if i >= 8:  # other half: all-or-nothing
                                    nc.scalar.activation(
                                        pt, sps, AF.Exp, bias=h2b)
                                else:
                                    jd = i - qc * 4  # diag col subtile
                                    if jd < 0:  # fully visible
                                        nc.scalar.activation(pt, sps, AF.Exp)
                                    else:
                                        if jd > 0:
                                            nc.vector.memset(
                                                pt[:, 0:jd * P].bitcast(f32),
                                                0.0)
                                        dsl = slice(jd * P, (jd + 1) * P)
                                        nc.scalar.activation(
                                            pt[:, dsl], sps[:, dsl], AF.Exp)
                                        nc.vector.tensor_mul(
                                            pt[:, dsl], pt[:, dsl], tril)
                                        if jd < 3:
                                            rsl = slice((jd + 1) * P, 512)
                                            nc.scalar.activation(
                                                pt[:, rsl], sps[:, rsl],
                                                AF.Exp)
                                nc.tensor.matmul(
                                    ctx, vaug[:, i, :], pt,
                                    start=(n == 0),
                                    stop=(n == len(kv_tiles) - 1))
                        # softmax normalization per head
                        for hh, ctx in ((0, ctx_e), (1, ctx_o)):
                            rr = ptp.tile([HD + 1, 512], f32, tag="rr")
                            rr0 = ptp.tile([1, 512], f32, tag="rr0")
                            bc = ptp.tile([HD, 512], f32, tag="bc")
                            nc.vector.reciprocal(
                                rr[HD:HD + 1, :], ctx[HD:HD + 1, :])
                            nc.sync.dma_start(rr0, rr[HD:HD + 1, :])
                            nc.gpsimd.partition_broadcast(bc, rr0)
                            if hh == 0:
                                nc.vector.tensor_mul(
                                    ctxT[0:HD, hp, qsl], ctx[0:HD, :], bc)
                            else:
                                tmp = ptp.tile([HD, 512], f32r, tag="tmp")
                                nc.vector.tensor_mul(tmp, ctx[0:HD, :], bc)
                                nc.sync.dma_start(ctxT[HD:P, hp, qsl], tmp)

        # ---------- Phase 3: Wo proj + residual -> x2 (DRAM) ----------
        with (
            tc.tile_pool(name="wop" + sfx, bufs=1) as wop,
            tc.tile_pool(name="ph3" + sfx, bufs=3) as ph3,
            tc.tile_pool(name="ps3" + sfx, bufs=2, space="PSUM") as ps3,
        ):
            wo_sb = wop.tile([P, DT, D], f32r)
            nc.sync.dma_start(
                wo_sb, wo_d.ap().rearrange("(pc p) n -> p pc n", p=P))
            bo_bc = wop.tile([P, D], f32)
            nc.sync.dma_start(bo_bc, bo_d.ap().to_broadcast([P, D]))
            for qt in range(NQ):
                xo_t = ph3.tile([P, D], f32, tag="xo_t")
                nc.sync.dma_start(xo_t, xl_d[qt * P:(qt + 1) * P, :])
                x2_t = ph3.tile([P, D], f32, tag="x2_t")
                for dc in range(2):
                    dsl = slice(dc * 512, (dc + 1) * 512)
                    acc = ps3.tile([P, 512], f32, tag="acc")
                    for pc in range(DT):
                        nc.tensor.matmul(
                            acc, ctxT[:, pc, qt * P:(qt + 1) * P],
                            wo_sb[:, pc, dsl],
                            start=(pc == 0), stop=(pc == DT - 1))
                    nc.vector.tensor_add(x2_t[:, dsl], acc, xo_t[:, dsl])
                    nc.vector.tensor_add(
                        x2_t[:, dsl], x2_t[:, dsl], bo_bc[:, dsl])
                nc.sync.dma_start(x2d[qt], x2_t)

    # ---------- Phase 4: LN2 + transpose ----------
    with tc.tile_pool(name="h2Tp" + sfx, bufs=1) as h2Tp:
        h2Tc = []
        for i in range(2):
            h2T_i = h2Tp.tile([P, DT, 512], fp8, tag=f"h2T{i}")
            h2Tc.append(h2T_i)
        with (
            tc.tile_pool(name="ln2" + sfx, bufs=5) as ln2,
            tc.tile_pool(name="ps4" + sfx, bufs=4, space="PSUM") as ps4,
        ):
            for qt in range(NQ):
                x2_t = ln2.tile([P, D], f32, tag="x2_t")
                nc.sync.dma_start(x2_t, x2d[qt])
                st = ln2.tile([P, 2, 6], f32, tag="st")
                nc.vector.bn_stats(st[:, 0, :], x2_t[:, 0:512])
                nc.vector.bn_stats(st[:, 1, :], x2_t[:, 512:1024])
                mv = ln2.tile([P, 2], f32, tag="mv")
                nc.vector.bn_aggr(mv, st)
                rstd = ln2.tile([P, 1], f32, tag="rstd")
                nc.scalar.activation(rstd, mv[:, 1:2], AF.Sqrt, bias=eps)
                nc.vector.reciprocal(rstd, rstd)
                nb = ln2.tile([P, 2], f32, tag="nb")
                nc.vector.tensor_scalar_mul(nb[:, 0:1], rstd, -1.0)
                nc.vector.tensor_mul(nb[:, 1:2], mv[:, 0:1], nb[:, 0:1])
                h2_t = ln2.tile([P, D], f32r, tag="h2_t")
                qpos = qt % 4
                for dh in range(2):
                    hsl = slice(dh * 512, (dh + 1) * 512)
                    nc.scalar.activation(
                        h2_t[:, hsl], x2_t[:, hsl], AF.Identity,
                        bias=nb[:, 1:2], scale=rstd)
                    tp = ps4.tile([P, 4, P], f32r, tag="tp")
                    for k in range(4):
                        dt = dh * 4 + k
                        nc.tensor.transpose(
                            tp[:, k, :],
                            h2_t[:, dt * P:(dt + 1) * P], ident)
                    dst = h2Tc[qt // 4][:, dh * 4:dh * 4 + 4,
                                        qpos * P:(qpos + 1) * P]
                    if dh == 0:
                        nc.scalar.copy(dst, tp)  # f32r -> fp8 convert
                    else:
                        nc.vector.tensor_copy(dst, tp)

        # ---------- Phase 5: FFN ----------
        with (
            tc.tile_pool(name="ffcst" + sfx, bufs=1) as ffcp,
            tc.tile_pool(name="ffw" + sfx, bufs=5) as ffw,
            tc.tile_pool(name="g1p" + sfx, bufs=1) as g1p,
            tc.tile_pool(name="ffo" + sfx, bufs=3) as ffo,
            tc.tile_pool(name="psa" + sfx, bufs=3, space="PSUM") as psa,
            tc.tile_pool(name="psf" + sfx, bufs=1, space="PSUM") as psf,
        ):
            b1f_sb = ffcp.tile([P, FT], f32)
            nc.sync.dma_start(b1f_sb, b1f_d[:, :])
            b2_bc = ffcp.tile([P, D], f32)
            nc.sync.dma_start(b2_bc, b2_d.ap().to_broadcast([P, D]))
            g1 = g1p.tile([P, FT, 512], fp8)
            for qc in range(2):
                qsl = slice(qc * 512, (qc + 1) * 512)
                # W1 + GELU for this q chunk, all ff chunks
                for fc in range(FT):
                    w1c = ffw.tile([P, DT, P], fp8, tag="w1c")
                    nc.sync.dma_start(w1c, w1_d[fc])
                    aps = psa.tile([P, 512], f32, tag="aps")
                    for dt in range(0, DT, 2):
                        nc.tensor.matmul(
                            aps, w1c[:, dt:dt + 2, :],
                            h2Tc[qc][:, dt:dt + 2, :],
                            start=(dt == 0), stop=(dt == DT - 2),
                            perf_mode=DR)
                    nc.scalar.activation(
                        g1[:, fc, :], aps, AF.Gelu,
                        bias=b1f_sb[:, fc:fc + 1], scale=1.0 / W1S)
                # W2 for this q chunk
                for dh in range(2):
                    dsl = slice(dh * 512, (dh + 1) * 512)
                    fps = []
                    for j in range(4):
                        fps_j = psf.tile([P, 512], f32, tag=f"fps{j}")
                        fps.append(fps_j)
                    for fc in range(FT // 2):
                        w2c = ffw.tile([P, 2, 512], fp8, tag="w2c")
                        nc.sync.dma_start(w2c, w2_d[fc][:, :, dsl])
                        for j in range(4):
                            nc.tensor.matmul(
                                fps[j],
                                g1[:, 2 * fc:2 * fc + 2, j * P:(j + 1) * P],
                                w2c, start=(fc == 0),
                                stop=(fc == FT // 2 - 1),
                                perf_mode=DR)
                    for j in range(4):
                        qt = qc * 4 + j
                        o_t = ffo.tile([P, 512], f32, tag="o_t")
                        x2s = ffo.tile([P, 512], f32, tag="x2s")
                        nc.sync.dma_start(x2s, x2d[qt, :, dsl])
                        nc.vector.scalar_tensor_tensor(
                            out=o_t, in0=fps[j], scalar=1.0 / W2S,
                            in1=x2s, op0=ALU.mult, op1=ALU.add)
                        nc.vector.tensor_add(o_t, o_t, b2_bc[:, dsl])
                        nc.sync.dma_start(
                            out_d[qt * P:(qt + 1) * P, dsl], o_t)


def _build_program(reps=1):
    nc = bacc.Bacc(None, target_bir_lowering=False)

    xl_d = nc.dram_tensor("xl", (T, D), f32, kind="ExternalInput")
    wq_d = nc.dram_tensor("wq", (D, D), f32r, kind="ExternalInput")
    wk_d = nc.dram_tensor("wk", (D, D), f32r, kind="ExternalInput")
    wv_d = nc.dram_tensor("wv", (D, D), f32r, kind="ExternalInput")
    wo_d = nc.dram_tensor("wo", (D, D), f32r, kind="ExternalInput")
    # w1: [fc, p, dt, m] pre-arranged fp8 (scaled by W1S)
    w1_d = nc.dram_tensor("w1", (FF // P, P, D // P, P), fp8,
                          kind="ExternalInput")
    # w2: [fc2, p, i, n] pre-arranged fp8 (scaled by W2S), i = k-pair
    w2_d = nc.dram_tensor("w2", (FF // (2 * P), P, 2, D), fp8,
                          kind="ExternalInput")
    qkvb_d = nc.dram_tensor("qkvb", (P, 3 * H // 2), f32, kind="ExternalInput")
    bo_d = nc.dram_tensor("bo_", (1, D), f32, kind="ExternalInput")
    b1f_d = nc.dram_tensor("b1f", (P, FF // P), f32, kind="ExternalInput")
    b2_d = nc.dram_tensor("b2_", (1, D), f32, kind="ExternalInput")
    h2b_d = nc.dram_tensor("h2b", (1, 1), f32, kind="ExternalInput")
    out_d = nc.dram_tensor("out", (TQ, D), f32, kind="ExternalOutput")

    NQ = TQ // P
    NT = T // P

    with tile.TileContext(nc) as tc:
        with (
            tc.tile_pool(name="const", bufs=1) as const,
            tc.tile_pool(name="dramp", bufs=1, space="DRAM") as dramp,
        ):
            ident_f = const.tile([P, P], f32)
            make_identity(nc, ident_f)
            ident = const.tile([P, P], f32r)
            nc.vector.tensor_copy(ident, ident_f)
            # S^T-space causal keep mask: keep where kv(part) <= q(free)
            tril_f = const.tile([P, P], f32)
            nc.gpsimd.memset(tril_f, 1.0)
            nc.gpsimd.affine_select(
                out=tril_f, in_=tril_f, compare_op=ALU.is_ge, fill=0.0,
                base=0, pattern=[[1, P]], channel_multiplier=-1,
            )
            tril = const.tile([P, P], f32r)
            nc.vector.tensor_copy(tril, tril_f)
            ones16 = const.tile([P, NT], f32)
            nc.vector.memset(ones16, 1.0)
            qkvb = const.tile([P, 3 * H // 2], f32)
            nc.sync.dma_start(qkvb, qkvb_d[:, :])
            eps = const.tile([P, 1], f32)
            nc.vector.memset(eps, 1e-5)
            h2b = const.tile([P, 1], f32)
            nc.sync.dma_start(h2b, h2b_d.ap().to_broadcast([P, 1]))

            x2d = dramp.tile([NQ, P, D], f32)  # x2 spill (post-attn residual)

            cst = (ident, tril, ones16, qkvb, eps, h2b)
            dram = (xl_d, wq_d, wk_d, wv_d, wo_d, w1_d, w2_d,
                    bo_d, b1f_d, b2_d, out_d)
            for rep in range(reps):
                sfx = f"r{rep}" if reps > 1 else ""
                _emit_body(nc, tc, sfx, cst, x2d, dram)

    nc.compile()
    return nc


def _prep_inputs(inputs):
    """Host-side: fold LN affine + score scale into weights; build per-core maps."""
    x = np.asarray(inputs["x"], dtype=np.float32)
    g1, b1_ = np.asarray(inputs["ln1_g"], np.float32), np.asarray(inputs["ln1_b"], np.float32)
    g2, b2_ = np.asarray(inputs["ln2_g"], np.float32), np.asarray(inputs["ln2_b"], np.float32)
    Wq = np.asarray(inputs["Wq"], np.float32)  # [H, D, HD]
    Wk = np.asarray(inputs["Wk"], np.float32)
    Wv = np.asarray(inputs["Wv"], np.float32)
    bq = np.asarray(inputs["bq"], np.float32)  # [H, HD]
    bk = np.asarray(inputs["bk"], np.float32)
    bv = np.asarray(inputs["bv"], np.float32)
    Wo = np.asarray(inputs["Wo"], np.float32)
    bo = np.asarray(inputs["bo"], np.float32)
    W1 = np.asarray(inputs["W1"], np.float32)
    b1 = np.asarray(inputs["b1"], np.float32)
    W2 = np.asarray(inputs["W2"], np.float32)
    b2 = np.asarray(inputs["b2"], np.float32)

    sc = 1.0 / np.sqrt(np.float32(HD))
    # [H, D, HD] -> [D, H*HD]
    wq_flat = np.transpose(Wq, (1, 0, 2)).reshape(D, D)
    wk_flat = np.transpose(Wk, (1, 0, 2)).reshape(D, D)
    wv_flat = np.transpose(Wv, (1, 0, 2)).reshape(D, D)
    wq_f = (g1[:, None] * wq_flat) * sc
    wk_f = g1[:, None] * wk_flat
    wv_f = g1[:, None] * wv_flat
    bq_f = (b1_ @ wq_flat + bq.reshape(D)) * sc
    bk_f = b1_ @ wk_flat + bk.reshape(D)
    bv_f = b1_ @ wv_flat + bv.reshape(D)
    # qkvb[0:64, 3*p+w] = even-head bias, [64:128, 3*p+w] = odd-head bias
    qkvb = np.zeros((P, 3 * H // 2), np.float32)
    for p_ in range(H // 2):
        for wi, bf in enumerate((bq_f, bk_f, bv_f)):
            qkvb[0:HD, 3 * p_ + wi] = bf[(2 * p_) * HD:(2 * p_ + 1) * HD]
            qkvb[HD:P, 3 * p_ + wi] = bf[(2 * p_ + 1) * HD:(2 * p_ + 2) * HD]

    w1_f = g2[:, None] * W1
    b1_f = (b2_ @ W1 + b1).reshape(FF // P, P).T.copy()  # [P, FF//P]

    import ml_dtypes

    e4m3 = ml_dtypes.float8_e4m3fn
    DT_, FT_ = D // P, FF // P
    # [fc, p, dt, m] layout; scaled into fp8's normal range
    w1_8 = np.clip(w1_f * W1S, -240, 240).astype(e4m3)
    w1_8 = w1_8.reshape(DT_, P, FT_, P).transpose(2, 1, 0, 3).copy()
    w2_8 = np.clip(W2 * W2S, -240, 240).astype(e4m3)
    w2_8 = w2_8.reshape(FT_ // 2, 2, P, D).transpose(0, 2, 1, 3).copy()

    shared = {
        "wq": np.ascontiguousarray(wq_f), "wk": np.ascontiguousarray(wk_f),
        "wv": np.ascontiguousarray(wv_f), "wo": np.ascontiguousarray(Wo),
        "w1": w1_8, "w2": w2_8,
        "qkvb": qkvb, "bo_": bo.reshape(1, D),
        "b1f": np.ascontiguousarray(b1_f), "b2_": b2.reshape(1, D),
    }
    in_maps = []
    for c in range(8):
        b, o = c // 2, c % 2
        own = x[b, o * TQ:(o + 1) * TQ]
        oth = x[b, (1 - o) * TQ:(2 - o) * TQ]
        m = dict(shared)
        m["xl"] = np.ascontiguousarray(np.concatenate([own, oth], axis=0))
        m["h2b"] = np.array([[0.0 if o == 1 else NEG]], np.float32)
        in_maps.append(m)
    return in_maps


def kernel(**inputs):
    if "nc" not in _CACHE:
        _CACHE["nc"] = _build_program()
    nc = _CACHE["nc"]
    in_maps = _prep_inputs(inputs)
    res = run_bass_kernel_spmd(nc, in_maps, core_ids=list(range(8)))
    out = np.empty((B, T, D), np.float32)
    for c in range(8):
        b, o = c // 2, c % 2
        out[b, o * TQ:(o + 1) * TQ] = res.results[c]["out"]
    return out


# revision 12
# speedup vs baseline: 68.3406x; 1.5761x over previous
"""Fused transformer block (nn_Block_2388001816768) on 8 Trainium2 NeuronCores.

Sharding: (batch, head-half) -> one core. Core c handles batch c//2 and
heads [8o, 8o+8) where o = c%2, over the FULL sequence. Causal attention
is exact (no masked-tile waste): q-chunk qc attends kv tiles 0..4qc+3
with a tril constant on the diagonal tile.

After the Wo projection each core holds a partial attention output
(its 8 heads' contribution) plus 0.5*(x + bo); a pairwise ReduceScatter
(add) between the two cores of a batch yields x2 = x + attn_out, split
so each core keeps its sequence half for LN2 + FFN.

Large matmuls run in float32r (TF32-like, full PE rate at free>=256),
fp32 accum. The FFN runs fp8e4 DoubleRow (2x PE rate): W1*16 / W2*64
are pre-scaled into fp8's normal range host-side; the inverse scales
fold into the GELU input scale and the output epilogue. LN scale/shift
and the 1/sqrt(HD) score scale are folded into projection weights
host-side.
"""

import numpy as np

import concourse.bacc as bacc
import concourse.bass as bass  # noqa: F401
import concourse.mybir as mybir
import concourse.tile as tile
from concourse.bass_utils import run_bass_kernel_spmd
from concourse.masks import make_identity

B, T, D, H = 4, 2048, 1024, 16
HD = D // H  # 64
FF = 4 * D  # 4096
TQ = T // 2  # output rows per core = 1024
P = 128
HL = H // 2  # heads per core = 8
HPL = HL // 2  # head pairs per core = 4

f32 = mybir.dt.float32
f32r = mybir.dt.float32r
fp8 = mybir.dt.float8e4
AF = mybir.ActivationFunctionType
ALU = mybir.AluOpType
DR = mybir.MatmulPerfMode.DoubleRow
W1S = 16.0  # host-side weight scale (fp8 range), undone by activation scale
W2S = 64.0

_CACHE = {}


def _emit_body(nc, tc, sfx, cst, x2d, p2d, dram):
    (xl_d, wq_d, wk_d, wv_d, wo_d, w1_d, w2_d, bo_d, b1f_d, b2_d, out_d) = dram
    ident, tril, ones16, qkvb, eps = cst

    DT = D // P  # 8 d-tiles
    NT = T // P  # 16 t-tiles
    NQ = TQ // P  # 8 own-half q-tiles
    FT = FF // P  # 32 ff-tiles

    with tc.tile_pool(name="ctxp" + sfx, bufs=1) as ctxp:
        ctxT = ctxp.tile([P, HPL, T], f32r)  # ctx^T head-pair-stacked

        with tc.tile_pool(name="hTp" + sfx, bufs=1) as hTp:
            # h^T in 4 t-chunks of 512 so phase 2 can overlap phase 1
            hTc = []
            for i in range(4):
                hT_i = hTp.tile([P, DT, 512], f32r, tag=f"hT{i}")
                hTc.append(hT_i)

            # ---------- Phase 1: LN1 + transpose (full T) ----------
            with (
                tc.tile_pool(name="ln1" + sfx, bufs=5) as ln1,
                tc.tile_pool(name="ps1" + sfx, bufs=4, space="PSUM") as ps1,
            ):
                for tt in range(NT):
                    x_t = ln1.tile([P, D], f32, tag="x_t")
                    nc.sync.dma_start(x_t, xl_d[tt * P:(tt + 1) * P, :])
                    st = ln1.tile([P, 2, 6], f32, tag="st")
                    nc.vector.bn_stats(st[:, 0, :], x_t[:, 0:512])
                    nc.vector.bn_stats(st[:, 1, :], x_t[:, 512:1024])
                    mv = ln1.tile([P, 2], f32, tag="mv")
                    nc.vector.bn_aggr(mv, st)
                    rstd = ln1.tile([P, 1], f32, tag="rstd")
                    nc.scalar.activation(rstd, mv[:, 1:2], AF.Sqrt, bias=eps)
                    nc.vector.reciprocal(rstd, rstd)
                    nb = ln1.tile([P, 2], f32, tag="nb")
                    nc.vector.tensor_scalar_mul(nb[:, 0:1], rstd, -1.0)
                    nc.vector.tensor_mul(
                        nb[:, 1:2], mv[:, 0:1], nb[:, 0:1])
                    h_t = ln1.tile([P, D], f32r, tag="h_t")
                    tpos = tt % 4
                    for dh in range(2):
                        hsl = slice(dh * 512, (dh + 1) * 512)
                        nc.scalar.activation(
                            h_t[:, hsl], x_t[:, hsl], AF.Identity,
                            bias=nb[:, 1:2], scale=rstd)
                        tp = ps1.tile([P, 4, P], f32r, tag="tp")
                        for k in range(4):
                            dt = dh * 4 + k
                            nc.tensor.transpose(
                                tp[:, k, :],
                                h_t[:, dt * P:(dt + 1) * P], ident)
                        dst = hTc[tt // 4][:, dh * 4:dh * 4 + 4,
                                           tpos * P:(tpos + 1) * P]
                        if dh == 0:
                            nc.scalar.copy(dst, tp)
                        else:
                            nc.vector.tensor_copy(dst, tp)

            # ---------- Phase 2: head-pair QKV + exact causal attn ----------
            # Local heads processed in pairs: even head in partitions 0:64,
            # odd head in 64:128 of shared pair tiles; scores use
            # tile_position row-split so the two K=64 matmuls share the PE
            # array.
            with (
                tc.tile_pool(name="whead" + sfx, bufs=1) as whead,
                tc.tile_pool(name="head" + sfx, bufs=2) as head,
                tc.tile_pool(name="pt" + sfx, bufs=3) as ptp,
                tc.tile_pool(name="psqkv" + sfx, bufs=2, space="PSUM") as psqkv,
                tc.tile_pool(name="pssc" + sfx, bufs=4, space="PSUM") as pssc,
                tc.tile_pool(name="psctx" + sfx, bufs=1, space="PSUM") as psctx,
            ):
                for hp in range(HPL):
                    wp = whead.tile([P, 3, DT, 2 * HD], f32r, tag="wp")
                    for wi, w_dram in enumerate((wq_d, wk_d, wv_d)):
                        nc.sync.dma_start(
                            wp[:, wi],
                            w_dram[:, hp * 2 * HD:(hp + 1) * 2 * HD]
                            .rearrange("(dt q) m -> q dt m", q=P))

                    qT = head.tile([P, T], f32r, tag="qT")
                    kT = head.tile([P, T], f32r, tag="kT")
                    vT = head.tile([P, T], f32r, tag="vT")
                    vaug_e = head.tile([P, NT, HD + 1], f32r, tag="vaug_e")
                    vaug_o = head.tile([P, NT, HD + 1], f32r, tag="vaug_o")
                    nc.vector.tensor_copy(
                        vaug_e[:, :, HD:HD + 1], ones16.unsqueeze(2))
                    nc.vector.tensor_copy(
                        vaug_o[:, :, HD:HD + 1], ones16.unsqueeze(2))
                    for (wi, dst, bcol) in (
                        (0, qT, 3 * hp + 0),
                        (1, kT, 3 * hp + 1),
                        (2, vT, 3 * hp + 2),
                    ):
                        for c in range(T // 512):
                            pp = psqkv.tile([P, 512], f32, tag="pp")
                            for dt in range(DT):
                                nc.tensor.matmul(
                                    pp, wp[:, wi, dt, :],
                                    hTc[c][:, dt, :],
                                    start=(dt == 0), stop=(dt == DT - 1))
                            nc.vector.tensor_scalar_add(
                                out=dst[:, c * 512:(c + 1) * 512],
                                in0=pp, scalar1=qkvb[:, bcol:bcol + 1])
                            if wi == 2:  # transpose V per chunk, inline
                                for kt in range(4 * c, 4 * c + 4):
                                    ksl = slice(kt * P, (kt + 1) * P)
                                    vp = pssc.tile([P, 512], f32r,
                                                   tag="sps")
                                    nc.tensor.transpose(
                                        vp[:, 0:P], vT[:, ksl], ident)
                                    nc.vector.tensor_copy(
                                        vaug_e[:, kt, 0:HD], vp[:, 0:HD])
                                    nc.vector.tensor_copy(
                                        vaug_o[:, kt, 0:HD], vp[:, HD:P])

                    for qc in range(4):
                        qsl = slice(qc * 512, (qc + 1) * 512)
                        kv_tiles = list(range(0, (qc + 1) * 4))
                        ctx_e = psctx.tile([HD + 1, 512], f32, tag="ctx_e")
                        ctx_o = psctx.tile([HD + 1, 512], f32, tag="ctx_o")
                        for n, i in enumerate(kv_tiles):
                            isl = slice(i * P, (i + 1) * P)
                            sps_e = pssc.tile([P, 512], f32, tag="sps")
                            nc.tensor.matmul(
                                sps_e, kT[0:HD, isl], qT[0:HD, qsl],
                                start=True, stop=True,
                                tile_position=(0, 0))
                            sps_o = pssc.tile([P, 512], f32, tag="sps")
                            nc.tensor.matmul(
                                sps_o, kT[HD:P, isl], qT[HD:P, qsl],
                                start=True, stop=True,
                                tile_position=(HD, 0))
                            for sps, vaug, ctx in (
                                (sps_e, vaug_e, ctx_e),
                                (sps_o, vaug_o, ctx_o),
                            ):
                                pt = ptp.tile([P, 512], f32r, tag="pt")
                                jd = i - qc * 4  # diag col subtile
                                if jd < 0:  # fully visible
                                    nc.scalar.activation(pt, sps, AF.Exp)
                                else:
                                    if jd > 0:
                                        nc.vector.memset(
                                            pt[:, 0:jd * P].bitcast(f32),
                                            0.0)
                                    dsl = slice(jd * P, (jd + 1) * P)
                                    nc.scalar.activation(
                                        pt[:, dsl], sps[:, dsl], AF.Exp)
                                    nc.vector.tensor_mul(
                                        pt[:, dsl], pt[:, dsl], tril)
                                    if jd < 3:
                                        rsl = slice((jd + 1) * P, 512)
                                        nc.scalar.activation(
                                            pt[:, rsl], sps[:, rsl],
                                            AF.Exp)
                                nc.tensor.matmul(
                                    ctx, vaug[:, i, :], pt,
                                    start=(n == 0),
                                    stop=(n == len(kv_tiles) - 1))
                        # softmax normalization per head
                        for hh, ctx in ((0, ctx_e), (1, ctx_o)):
                            rr = ptp.tile([HD + 1, 512], f32, tag="rr")
                            rr0 = ptp.tile([1, 512], f32, tag="rr0")
                            bc = ptp.tile([HD, 512], f32, tag="bc")
                            nc.vector.reciprocal(
                                rr[HD:HD + 1, :], ctx[HD:HD + 1, :])
                            nc.sync.dma_start(rr0, rr[HD:HD + 1, :])
                            nc.gpsimd.partition_broadcast(bc, rr0)
                            if hh == 0:
                                nc.vector.tensor_mul(
                                    ctxT[0:HD, hp, qsl], ctx[0:HD, :], bc)
                            else:
                                tmp = ptp.tile([HD, 512], f32r, tag="tmp")
                                nc.vector.tensor_mul(tmp, ctx[0:HD, :], bc)
                                nc.sync.dma_start(ctxT[HD:P, hp, qsl], tmp)

        # ---------- Phase 3: partial Wo + 0.5*(x+bo) -> p2 (DRAM) ----------
        with (
            tc.tile_pool(name="wop" + sfx, bufs=1) as wop,
            tc.tile_pool(name="ph3" + sfx, bufs=3) as ph3,
            tc.tile_pool(name="ps3" + sfx, bufs=2, space="PSUM") as ps3,
        ):
            wo_sb = wop.tile([P, HPL, D], f32r)
            nc.sync.dma_start(
                wo_sb, wo_d.ap().rearrange("(pc p) n -> p pc n", p=P))
            bo_bc = wop.tile([P, D], f32)  # pre-halved host-side
            nc.sync.dma_start(bo_bc, bo_d.ap().to_broadcast([P, D]))
            # write order interleaves halves so the first ReduceScatter can
            # launch after 8 tiles
            order = [0, 8, 1, 9, 2, 10, 3, 11, 4, 12, 5, 13, 6, 14, 7, 15]
            for nq, qt in enumerate(order):
                xo_t = ph3.tile([P, D], f32, tag="xo_t")
                nc.sync.dma_start(xo_t, xl_d[qt * P:(qt + 1) * P, :])
                x2_t = ph3.tile([P, D], f32, tag="x2_t")
                for dc in range(2):
                    dsl = slice(dc * 512, (dc + 1) * 512)
                    acc = ps3.tile([P, 512], f32, tag="acc")
                    for pc in range(HPL):
                        nc.tensor.matmul(
                            acc, ctxT[:, pc, qt * P:(qt + 1) * P],
                            wo_sb[:, pc, dsl],
                            start=(pc == 0), stop=(pc == HPL - 1))
                    nc.vector.scalar_tensor_tensor(
                        out=x2_t[:, dsl], in0=xo_t[:, dsl], scalar=0.5,
                        in1=acc, op0=ALU.mult, op1=ALU.add)
                    nc.vector.tensor_add(
                        x2_t[:, dsl], x2_t[:, dsl], bo_bc[:, dsl])
                nc.sync.dma_start(p2d[(qt % 8) // 4][qt // 8, qt % 4], x2_t)
                if nq == 7:
                    nc.gpsimd.collective_compute(
                        "ReduceScatter", ALU.add,
                        replica_groups=[[0, 1], [2, 3], [4, 5], [6, 7]],
                        ins=[p2d[0].opt()], outs=[x2d[0:4].opt()])
            nc.gpsimd.collective_compute(
                "ReduceScatter", ALU.add,
                replica_groups=[[0, 1], [2, 3], [4, 5], [6, 7]],
                ins=[p2d[1].opt()], outs=[x2d[4:8].opt()])

    # ---------- Phase 4: LN2 + transpose (own half) ----------
    with tc.tile_pool(name="h2Tp" + sfx, bufs=1) as h2Tp:
        h2Tc = []
        for i in range(2):
            h2T_i = h2Tp.tile([P, DT, 512], fp8, tag=f"h2T{i}")
            h2Tc.append(h2T_i)
        with (
            tc.tile_pool(name="ln2" + sfx, bufs=5) as ln2,
            tc.tile_pool(name="ps4" + sfx, bufs=4, space="PSUM") as ps4,
        ):
            for qt in range(NQ):
                x2_t = ln2.tile([P, D], f32, tag="x2_t")
                nc.sync.dma_start(x2_t, x2d[qt])
                st = ln2.tile([P, 2, 6], f32, tag="st")
                nc.vector.bn_stats(st[:, 0, :], x2_t[:, 0:512])
                nc.vector.bn_stats(st[:, 1, :], x2_t[:, 512:1024])
                mv = ln2.tile([P, 2], f32, tag="mv")
                nc.vector.bn_aggr(mv, st)
                rstd = ln2.tile([P, 1], f32, tag="rstd")
                nc.scalar.activation(rstd, mv[:, 1:2], AF.Sqrt, bias=eps)
                nc.vector.reciprocal(rstd, rstd)
                nb = ln2.tile([P, 2], f32, tag="nb")
                nc.vector.tensor_scalar_mul(nb[:, 0:1], rstd, -1.0)
                nc.vector.tensor_mul(nb[:, 1:2], mv[:, 0:1], nb[:, 0:1])
                h2_t = ln2.tile([P, D], f32r, tag="h2_t")
                qpos = qt % 4
                for dh in range(2):
                    hsl = slice(dh * 512, (dh + 1) * 512)
                    nc.scalar.activation(
                        h2_t[:, hsl], x2_t[:, hsl], AF.Identity,
                        bias=nb[:, 1:2], scale=rstd)
                    tp = ps4.tile([P, 4, P], f32r, tag="tp")
                    for k in range(4):
                        dt = dh * 4 + k
                        nc.tensor.transpose(
                            tp[:, k, :],
                            h2_t[:, dt * P:(dt + 1) * P], ident)
                    dst = h2Tc[qt // 4][:, dh * 4:dh * 4 + 4,
                                        qpos * P:(qpos + 1) * P]
                    if dh == 0:
                        nc.scalar.copy(dst, tp)  # f32r -> fp8 convert
                    else:
                        nc.vector.tensor_copy(dst, tp)

        # ---------- Phase 5: FFN (fp8 DoubleRow) ----------
        with (
            tc.tile_pool(name="ffcst" + sfx, bufs=1) as ffcp,
            tc.tile_pool(name="ffw" + sfx, bufs=5) as ffw,
            tc.tile_pool(name="g1p" + sfx, bufs=1) as g1p,
            tc.tile_pool(name="ffo" + sfx, bufs=3) as ffo,
            tc.tile_pool(name="psa" + sfx, bufs=3, space="PSUM") as psa,
            tc.tile_pool(name="psf" + sfx, bufs=1, space="PSUM") as psf,
        ):
            b1f_sb = ffcp.tile([P, FT], f32)
            nc.sync.dma_start(b1f_sb, b1f_d[:, :])
            b2_bc = ffcp.tile([P, D], f32)
            nc.sync.dma_start(b2_bc, b2_d.ap().to_broadcast([P, D]))
            g1 = g1p.tile([P, FT, 512], fp8)
            for qc in range(2):
                qsl = slice(qc * 512, (qc + 1) * 512)
                # W1 + GELU for this q chunk, all ff chunks
                for fc in range(FT):
                    w1c = ffw.tile([P, DT, P], fp8, tag="w1c")
                    nc.sync.dma_start(w1c, w1_d[fc])
                    aps = psa.tile([P, 512], f32, tag="aps")
                    for dt in range(0, DT, 2):
                        nc.tensor.matmul(
                            aps, w1c[:, dt:dt + 2, :],
                            h2Tc[qc][:, dt:dt + 2, :],
                            start=(dt == 0), stop=(dt == DT - 2),
                            perf_mode=DR)
                    nc.scalar.activation(
                        g1[:, fc, :], aps, AF.Gelu,
                        bias=b1f_sb[:, fc:fc + 1], scale=1.0 / W1S)
                # W2 for this q chunk
                for dh in range(2):
                    dsl = slice(dh * 512, (dh + 1) * 512)
                    fps = []
                    for j in range(4):
                        fps_j = psf.tile([P, 512], f32, tag=f"fps{j}")
                        fps.append(fps_j)
                    for fc in range(FT // 2):
                        w2c = ffw.tile([P, 2, 512], fp8, tag="w2c")
                        nc.sync.dma_start(w2c, w2_d[fc][:, :, dsl])
                        for j in range(4):
                            nc.tensor.matmul(
                                fps[j],
                                g1[:, 2 * fc:2 * fc + 2, j * P:(j + 1) * P],
                                w2c, start=(fc == 0),
                                stop=(fc == FT // 2 - 1),
                                perf_mode=DR)
                    for j in range(4):
                        qt = qc * 4 + j
                        o_t = ffo.tile([P, 512], f32, tag="o_t")
                        x2s = ffo.tile([P, 512], f32, tag="x2s")
                        nc.sync.dma_start(x2s, x2d[qt, :, dsl])
                        nc.vector.scalar_tensor_tensor(
                            out=o_t, in0=fps[j], scalar=1.0 / W2S,
                            in1=x2s, op0=ALU.mult, op1=ALU.add)
                        nc.vector.tensor_add(o_t, o_t, b2_bc[:, dsl])
                        nc.sync.dma_start(
                            out_d[qt * P:(qt + 1) * P, dsl], o_t)


def _build_program(reps=1):
    nc = bacc.Bacc(None, target_bir_lowering=False)

    xl_d = nc.dram_tensor("xl", (T, D), f32, kind="ExternalInput")
    wq_d = nc.dram_tensor("wq", (D, HL * HD), f32r, kind="ExternalInput")
    wk_d = nc.dram_tensor("wk", (D, HL * HD), f32r, kind="ExternalInput")
    wv_d = nc.dram_tensor("wv", (D, HL * HD), f32r, kind="ExternalInput")
    wo_d = nc.dram_tensor("wo", (HL * HD, D), f32r, kind="ExternalInput")
    # w1: [fc, p, dt, m] pre-arranged fp8 (scaled by W1S)
    w1_d = nc.dram_tensor("w1", (FF // P, P, D // P, P), fp8,
                          kind="ExternalInput")
    # w2: [fc2, p, i, n] pre-arranged fp8 (scaled by W2S), i = k-pair
    w2_d = nc.dram_tensor("w2", (FF // (2 * P), P, 2, D), fp8,
                          kind="ExternalInput")
    qkvb_d = nc.dram_tensor("qkvb", (P, 3 * HPL), f32, kind="ExternalInput")
    bo_d = nc.dram_tensor("bo_", (1, D), f32, kind="ExternalInput")
    b1f_d = nc.dram_tensor("b1f", (P, FF // P), f32, kind="ExternalInput")
    b2_d = nc.dram_tensor("b2_", (1, D), f32, kind="ExternalInput")
    out_d = nc.dram_tensor("out", (TQ, D), f32, kind="ExternalOutput")

    NQ = TQ // P
    NT = T // P

    with tile.TileContext(nc) as tc:
        with (
            tc.tile_pool(name="const", bufs=1) as const,
            tc.tile_pool(name="dramp", bufs=1, space="DRAM") as dramp,
        ):
            ident_f = const.tile([P, P], f32)
            make_identity(nc, ident_f)
            ident = const.tile([P, P], f32r)
            nc.vector.tensor_copy(ident, ident_f)
            # S^T-space causal keep mask: keep where kv(part) <= q(free)
            tril_f = const.tile([P, P], f32)
            nc.gpsimd.memset(tril_f, 1.0)
            nc.gpsimd.affine_select(
                out=tril_f, in_=tril_f, compare_op=ALU.is_ge, fill=0.0,
                base=0, pattern=[[1, P]], channel_multiplier=-1,
            )
            tril = const.tile([P, P], f32r)
            nc.vector.tensor_copy(tril, tril_f)
            ones16 = const.tile([P, NT], f32)
            nc.vector.memset(ones16, 1.0)
            qkvb = const.tile([P, 3 * HPL], f32)
            nc.sync.dma_start(qkvb, qkvb_d[:, :])
            eps = const.tile([P, 1], f32)
            nc.vector.memset(eps, 1e-5)

            x2d = dramp.tile([NQ, P, D], f32)  # post-RS own-half residual
            # partial attn out, one contiguous [half, tile] buffer per
            # ReduceScatter chunk
            p2da = dramp.tile([2, 4, P, D], f32, tag="p2a")
            p2db = dramp.tile([2, 4, P, D], f32, tag="p2b")
            p2d = (p2da, p2db)

            cst = (ident, tril, ones16, qkvb, eps)
            dram = (xl_d, wq_d, wk_d, wv_d, wo_d, w1_d, w2_d,
                    bo_d, b1f_d, b2_d, out_d)
            for rep in range(reps):
                sfx = f"r{rep}" if reps > 1 else ""
                _emit_body(nc, tc, sfx, cst, x2d, p2d, dram)

    nc.compile()
    return nc


def _prep_inputs(inputs):
    """Host-side: fold LN affine + score scale into weights; build per-core maps."""
    x = np.asarray(inputs["x"], dtype=np.float32)
    g1, b1_ = np.asarray(inputs["ln1_g"], np.float32), np.asarray(inputs["ln1_b"], np.float32)
    g2, b2_ = np.asarray(inputs["ln2_g"], np.float32), np.asarray(inputs["ln2_b"], np.float32)
    Wq = np.asarray(inputs["Wq"], np.float32)  # [H, D, HD]
    Wk = np.asarray(inputs["Wk"], np.float32)
    Wv = np.asarray(inputs["Wv"], np.float32)
    bq = np.asarray(inputs["bq"], np.float32)  # [H, HD]
    bk = np.asarray(inputs["bk"], np.float32)
    bv = np.asarray(inputs["bv"], np.float32)
    Wo = np.asarray(inputs["Wo"], np.float32)
    bo = np.asarray(inputs["bo"], np.float32)
    W1 = np.asarray(inputs["W1"], np.float32)
    b1 = np.asarray(inputs["b1"], np.float32)
    W2 = np.asarray(inputs["W2"], np.float32)
    b2 = np.asarray(inputs["b2"], np.float32)

    sc = 1.0 / np.sqrt(np.float32(HD))
    # [H, D, HD] -> [D, H*HD]
    wq_flat = np.transpose(Wq, (1, 0, 2)).reshape(D, D)
    wk_flat = np.transpose(Wk, (1, 0, 2)).reshape(D, D)
    wv_flat = np.transpose(Wv, (1, 0, 2)).reshape(D, D)
    wq_f = (g1[:, None] * wq_flat) * sc
    wk_f = g1[:, None] * wk_flat
    wv_f = g1[:, None] * wv_flat
    bq_f = (b1_ @ wq_flat + bq.reshape(D)) * sc
    bk_f = b1_ @ wk_flat + bk.reshape(D)
    bv_f = b1_ @ wv_flat + bv.reshape(D)

    w1_f = g2[:, None] * W1
    b1_f = (b2_ @ W1 + b1).reshape(FF // P, P).T.copy()  # [P, FF//P]

    import ml_dtypes

    e4m3 = ml_dtypes.float8_e4m3fn
    DT_, FT_ = D // P, FF // P
    # [fc, p, dt, m] layout; scaled into fp8's normal range
    w1_8 = np.clip(w1_f * W1S, -240, 240).astype(e4m3)
    w1_8 = w1_8.reshape(DT_, P, FT_, P).transpose(2, 1, 0, 3).copy()
    w2_8 = np.clip(W2 * W2S, -240, 240).astype(e4m3)
    w2_8 = w2_8.reshape(FT_ // 2, 2, P, D).transpose(0, 2, 1, 3).copy()

    shared = {
        "w1": w1_8, "w2": w2_8,
        "bo_": np.ascontiguousarray(0.5 * bo.reshape(1, D)),
        "b1f": np.ascontiguousarray(b1_f), "b2_": b2.reshape(1, D),
    }
    # per head-half: weight column/row slices + bias table
    half = {}
    for o in range(2):
        hsl = slice(o * HL * HD, (o + 1) * HL * HD)
        qkvb = np.zeros((P, 3 * HPL), np.float32)
        for hp in range(HPL):
            gp = o * HPL + hp  # global head pair
            for wi, bf in enumerate((bq_f, bk_f, bv_f)):
                qkvb[0:HD, 3 * hp + wi] = bf[(2 * gp) * HD:(2 * gp + 1) * HD]
                qkvb[HD:P, 3 * hp + wi] = bf[(2 * gp + 1) * HD:(2 * gp + 2) * HD]
        half[o] = {
            "wq": np.ascontiguousarray(wq_f[:, hsl]),
            "wk": np.ascontiguousarray(wk_f[:, hsl]),
            "wv": np.ascontiguousarray(wv_f[:, hsl]),
            "wo": np.ascontiguousarray(Wo[hsl, :]),
            "qkvb": qkvb,
        }
    in_maps = []
    for c in range(8):
        b, o = c // 2, c % 2
        m = dict(shared)
        m.update(half[o])
        m["xl"] = np.ascontiguousarray(x[b])
        in_maps.append(m)
    return in_maps


def kernel(**inputs):
    if "nc" not in _CACHE:
        _CACHE["nc"] = _build_program()
    nc = _CACHE["nc"]
    in_maps = _prep_inputs(inputs)
    res = run_bass_kernel_spmd(nc, in_maps, core_ids=list(range(8)))
    out = np.empty((B, T, D), np.float32)
    for c in range(8):
        b, o = c // 2, c % 2
        out[b, o * TQ:(o + 1) * TQ] = res.results[c]["out"]
    return out
